# revision 1
# baseline (speedup 1.0000x reference)
"""Single-head causal attention (B=8, T=2048, C=1024, H=128) on 8 TRN2 NeuronCores.

Sharding: data-parallel over batch — core b computes batch element b entirely
(no collectives). Host pre-transposes x[b] to xT=[C,T] so every matmul has its
contraction dim on SBUF partitions; the device returns out^T=[H,T] which the
host transposes back.

Per-core dataflow (T split into 4 chunks of 512 columns):
  qT/kT/vT chunk = sum_c W[c-tile].T @ xT[c-tile]  (float32r matmuls, +bias)
  v_nat[s-tile]  = PE-transpose of vT slices, cast bf16
  scoresT[s,t]   = kT-tile.T-as-lhsT @ qT-chunk    (float32r, causal blocks only)
  expT           = Exp(scoresT) on ACT -> bf16, diagonal blocks masked
  out^T         += v_nat[i] @ expT ; denom += ones @ expT   (bf16 matmuls)
  out^T[:,chunk] = out^T * 1/denom  -> DMA out
"""

import os
import numpy as np

T, C, H = 2048, 1024, 128
B = 8
P = 128
CT = C // P          # 8 contraction tiles
NCH = 4              # t-chunks
CHW = T // NCH       # 512 chunk width
SPC = CHW // P       # 4 s-tiles per chunk
N_CORES = 8

LAST_EXEC_TIME_NS = None

_BUILT = None


def _build():
    global _BUILT
    if _BUILT is not None:
        return _BUILT

    import concourse.bass as bass  # noqa: F401
    import concourse.mybir as mybir
    from concourse import bacc
    from concourse.tile import TileContext

    F32 = mybir.dt.float32
    F32R = mybir.dt.float32r
    BF16 = mybir.dt.bfloat16
    Identity = mybir.ActivationFunctionType.Identity
    Exp = mybir.ActivationFunctionType.Exp

    nc = bacc.Bacc()

    xT_ext = nc.declare_dram_parameter("xT", [C, T], F32R, isOutput=False)
    w_ext = {
        n: nc.declare_dram_parameter(n, [C, H], F32R, isOutput=False)
        for n in ("Wq", "Wk", "Wv")
    }
    b_ext = {
        n: nc.declare_dram_parameter(n, [H, 1], F32, isOutput=False)
        for n in ("bq", "bk", "bv")
    }
    ident_ext = nc.declare_dram_parameter("ident", [P, P], F32, isOutput=False)
    masks_ext = nc.declare_dram_parameter("masks", [P, SPC, CHW], BF16, isOutput=False)
    out_ext = nc.declare_dram_parameter("out", [H, T], F32, isOutput=True)

    xT_r = xT_ext.rearrange("(ct p) t -> p ct t", p=P)
    w_r = {n: w_ext[n].rearrange("(ct p) h -> p ct h", p=P) for n in w_ext}

    with TileContext(nc) as tc:
        with (
            tc.tile_pool(name="const", bufs=1) as const,
            tc.tile_pool(name="kt", bufs=16) as kt_pool,
            tc.tile_pool(name="vnat", bufs=16) as v_pool,
            tc.tile_pool(name="xch", bufs=2) as x_pool,
            tc.tile_pool(name="qv", bufs=2) as qv_pool,
            tc.tile_pool(name="ex", bufs=8) as e_pool,
            tc.tile_pool(name="outp", bufs=2) as out_pool,
            tc.tile_pool(name="ps_proj", bufs=2, space="PSUM") as proj_ps,
            tc.tile_pool(name="ps_sc", bufs=2, space="PSUM") as sc_ps,
            tc.tile_pool(name="ps_o", bufs=2, space="PSUM") as o_ps,
            tc.tile_pool(name="ps_d", bufs=1, space="PSUM") as d_ps,
            tc.tile_pool(name="ps_tr", bufs=1, space="PSUM") as tr_ps,
        ):
            w_sb = {}
            b_sb = {}
            # Wq c0 + x0 c0 gate the first matmul: order them first.
            for n in ("Wq", "Wk", "Wv"):
                w_sb[n] = [
                    const.tile([P, H], F32R, tag=f"w_{n}_{c}", name=f"w_{n}_{c}")
                    for c in range(CT)
                ]
            for n in ("bq", "bk", "bv"):
                b_sb[n] = const.tile([H, 1], F32, tag=f"b_{n}", name=f"b_{n}")
            x0_tiles = []
            for c in range(CT):
                nc.sync.dma_start(w_sb["Wq"][c][:], w_r["Wq"][:, c, :])
                xt = x_pool.tile([P, CHW], F32R, tag=f"xc{c}", name=f"x0_{c}")
                nc.sync.dma_start(xt[:], xT_r[:, c, 0:CHW])
                x0_tiles.append(xt)
            nc.sync.dma_start(b_sb["bq"][:], b_ext["bq"][:])
            for n, bn in (("Wk", "bk"), ("Wv", "bv")):
                for c in range(CT):
                    nc.sync.dma_start(w_sb[n][c][:], w_r[n][:, c, :])
                nc.sync.dma_start(b_sb[bn][:], b_ext[bn][:])
            ident = const.tile([P, P], F32, tag="ident")
            nc.sync.dma_start(ident[:], ident_ext[:])
            masks = const.tile([P, SPC, CHW], BF16, tag="masks")
            nc.sync.dma_start(masks[:], masks_ext[:])
            ones_bf = const.tile([P, P], BF16, tag="ones")
            nc.vector.memset(ones_bf[:], 1.0)

            # PE warmup: dummy matmuls spanning the DMA prologue so HAM is at
            # full clock when the first real matmul issues.
            warm_src = const.tile([P, CHW], BF16, tag="warm_src")
            nc.vector.memset(warm_src[:], 0.0)
            ps_warm = sc_ps.tile([P, CHW], F32, tag="sc", name="ps_warm")
            for _w in range(44):
                nc.tensor.matmul(
                    ps_warm[:], ones_bf[:], warm_src[:], start=True, stop=True,
                )

            kt_tiles = [None] * (NCH * SPC)
            v_tiles = [None] * (NCH * SPC)

            for j in range(NCH):
                tsl = slice(CHW * j, CHW * (j + 1))

                # ---- load x chunk (split per c-tile for fine-grained deps) ----
                if j == 0:
                    x_tiles = x0_tiles
                else:
                    x_tiles = []
                    for c in range(CT):
                        xt = x_pool.tile([P, CHW], F32R, tag=f"xc{c}", name=f"x{j}_{c}")
                        nc.sync.dma_start(xt[:], xT_r[:, c, tsl])
                        x_tiles.append(xt)

                # ---- projections (float32r) ----
                q_ch = qv_pool.tile([P, CHW], F32R, tag="qch")
                v_ch = qv_pool.tile([P, CHW], F32, tag="vch")

                for name, wname, bname in (
                    ("q", "Wq", "bq"),
                    ("k", "Wk", "bk"),
                    ("v", "Wv", "bv"),
                ):
                    ps = proj_ps.tile([P, CHW], F32, tag="proj")
                    for c in range(CT):
                        nc.tensor.matmul(
                            ps[:],
                            w_sb[wname][c][:],
                            x_tiles[c][:],
                            start=(c == 0),
                            stop=(c == CT - 1),
                        )
                    if name == "q":
                        nc.scalar.activation(q_ch[:], ps[:], Identity, bias=b_sb[bname][:])
                    elif name == "v":
                        nc.scalar.activation(v_ch[:], ps[:], Identity, bias=b_sb[bname][:])
                    else:
                        for st in range(SPC):
                            i_g = SPC * j + st
                            ktile = kt_pool.tile([P, P], F32R, tag="kt", name=f"kt_{i_g}")
                            nc.scalar.activation(
                                ktile[:],
                                ps[:, P * st : P * (st + 1)],
                                Identity,
                                bias=b_sb[bname][:],
                            )
                            kt_tiles[i_g] = ktile

                # ---- transpose v chunk to natural [s, h] bf16 tiles ----
                for st in range(SPC):
                    i_g = SPC * j + st
                    ps_t = tr_ps.tile([P, P], F32, tag="tr")
                    nc.tensor.transpose(ps_t[:], v_ch[:, P * st : P * (st + 1)], ident[:])
                    vt = v_pool.tile([P, P], BF16, tag="vnat", name=f"vnat_{i_g}")
                    nc.vector.tensor_copy(vt[:], ps_t[:])
                    v_tiles[i_g] = vt

                # ---- attention for this chunk ----
                n_s = SPC * (j + 1)
                ps_o = o_ps.tile([P, CHW], F32, tag="o")
                ps_d = d_ps.tile([P, CHW], F32, tag="d")
                for i in range(n_s):
                    ps_sc = sc_ps.tile([P, CHW], F32, tag="sc")
                    nc.tensor.matmul(
                        ps_sc[:],
                        kt_tiles[i][:],
                        q_ch[:],
                        start=True,
                        stop=True,
                    )
                    eb = e_pool.tile([P, CHW], BF16, tag="e")
                    nc.scalar.activation(eb[:], ps_sc[:], Exp)
                    if i >= SPC * j:
                        em = e_pool.tile([P, CHW], BF16, tag="em")
                        nc.vector.tensor_tensor(
                            em[:], eb[:], masks[:, i - SPC * j, :], mybir.AluOpType.mult
                        )
                    else:
                        em = eb
                    nc.tensor.matmul(
                        ps_d[:], ones_bf[:], em[:],
                        start=(i == 0), stop=(i == n_s - 1),
                    )
                    nc.tensor.matmul(
                        ps_o[:], v_tiles[i][:], em[:],
                        start=(i == 0), stop=(i == n_s - 1),
                    )

                recip = out_pool.tile([P, CHW], F32, tag="recip")
                nc.vector.reciprocal_approx_fast(out=recip[:], in_=ps_d[:])
                o_sb = out_pool.tile([P, CHW], F32, tag="osb")
                nc.vector.tensor_tensor(o_sb[:], ps_o[:], recip[:], mybir.AluOpType.mult)
                nc.sync.dma_start(out_ext[:, tsl], o_sb[:])

    nc.compile()
    _BUILT = nc
    return nc


def _host_inputs(x, Wq, bq, Wk, bk, Wv, bv):
    ident = np.eye(P, dtype=np.float32)
    # masks[p, r, ft] = 1 if p <= ft - 128*r else 0  (keep s_global <= t_global)
    import ml_dtypes
    ps = np.arange(P)[:, None, None]
    r = np.arange(SPC)[None, :, None]
    ft = np.arange(CHW)[None, None, :]
    masks = (ps <= ft - P * r).astype(ml_dtypes.bfloat16)

    shared = {
        "Wq": np.ascontiguousarray(Wq, dtype=np.float32),
        "Wk": np.ascontiguousarray(Wk, dtype=np.float32),
        "Wv": np.ascontiguousarray(Wv, dtype=np.float32),
        "bq": np.ascontiguousarray(bq, dtype=np.float32).reshape(H, 1),
        "bk": np.ascontiguousarray(bk, dtype=np.float32).reshape(H, 1),
        "bv": np.ascontiguousarray(bv, dtype=np.float32).reshape(H, 1),
        "ident": ident,
        "masks": masks,
    }
    in_maps = []
    for b in range(B):
        m = dict(shared)
        m["xT"] = np.ascontiguousarray(np.asarray(x[b], dtype=np.float32).T)
        in_maps.append(m)
    return in_maps


def kernel(x, Wq, bq, Wk, bk, Wv, bv):
    global LAST_EXEC_TIME_NS
    from concourse.bass_utils import run_bass_kernel_spmd

    nc = _build()
    in_maps = _host_inputs(x, Wq, bq, Wk, bk, Wv, bv)
    trace = os.environ.get("BASS_ATTN_TRACE", "0") == "1"
    res = run_bass_kernel_spmd(nc, in_maps, core_ids=list(range(N_CORES)), trace=trace)
    LAST_EXEC_TIME_NS = res.exec_time_ns
    out = np.stack([res.results[b]["out"].T for b in range(B)], axis=0)
    return np.ascontiguousarray(out, dtype=np.float32)

